# revision 1
# baseline (speedup 1.0000x reference)
"""Trainium2 Bass kernel for nn_MemoryNetwork (scatter_memory).

Computation (reference, per batch row b):
    f = feature / ||feature||                       [B, 768]
    topic = f @ W_topic.T ; dom = f @ W_domain.T    [B, 256]
    att   = softmax_m(TAU * topic . memory[d,m])    [B, 9, 10]
    sep   = sum_m att * memory[d,m]                 [B, 9, 256]
    out   = softmax_d(TAU * sep . dom)              [B, 1, 9]

Reformulation: the memory banks are tiny, so fold them into the projection
weights on the host:
    P = mem_flat @ W_topic ; Q = mem_flat @ W_domain ; R = [P; Q]  [180, 768]
Per row only one [768 x 180] product is needed:
    raw    = feature @ R.T                   (rawS | rawT)
    r      = TAU / ||feature||
    ex     = exp(rawS * r - SHIFT)           (softmax_m numerator, const shift
                                              instead of max-subtraction; safe:
                                              logits are in [-130, 110])
    sums_d = sum_m ex ; wsum_d = sum_m ex * rawT
    datt   = (wsum / sums) * r               (= TAU * domain_att)
    out    = softmax_d(datt)                 (const shift again)

Precision/speed: the PE cannot amortize fp32 weight loads (each fp32 matmul
self-loads its stationary twice at ~260ns), so fp32 matmuls measure ~3x
slower than their streaming cost. Instead the matmul runs as a compensated
fp16 pair: f = fhi + flo, R = Rhi + Rlo (exact fp16 splits, done host-side),
raw = fhi@Rhi + fhi@Rlo + flo@Rhi accumulated in fp32 PSUM -- ~20-bit
effective mantissa, measured ~2e-4 absmax output error vs the fp32
reference. Same DMA bytes as fp32 (2 x fp16 planes).

Sharding: data-parallel over B across 8 cores (4096 rows each). Features are
sent transposed [768, 4096] so matmuls contract over partitions directly;
row norms (r = TAU/||f||) ride along from the same host pass.
"""

import sys

sys.path.insert(0, "/opt/trn_rl_repo")

import numpy as np

B, IN, E, D, M = 32768, 768, 256, 9, 10
NCORES = 8
BC = B // NCORES  # rows per core
P = 128           # partition tile
NT = BC // P      # batch tiles per core (32)
G = 8             # tiles per softmax group
NG = NT // G
DM = 2 * D * M    # 180
KC = IN // P      # contraction chunks (6)
TAU = 32.0
SHIFT = 50.0

_CACHE: dict = {}


def _build_nc(repeat=1):
    from contextlib import ExitStack

    import concourse.bacc as bacc
    import concourse.tile as tile
    from concourse import mybir

    F32 = mybir.dt.float32
    F16 = mybir.dt.float16
    AF = mybir.ActivationFunctionType

    nc = bacc.Bacc(trn_type="TRN2")
    fhi = nc.dram_tensor("fhi", [IN, BC], F16, kind="ExternalInput")
    flo = nc.dram_tensor("flo", [IN, BC], F16, kind="ExternalInput")
    # rt2[k] columns 0:180 = Rhi[k], 180:360 = Rlo[k]
    rt2 = nc.dram_tensor("rt2", [IN, 2 * DM], F16, kind="ExternalInput")
    rin = nc.dram_tensor("rin", [P, NT], F32, kind="ExternalInput")
    out = nc.dram_tensor("out", [BC, D], F32, kind="ExternalOutput")

    LB = 4 * P  # feature DMA block: 4 batch tiles per transfer
    with tile.TileContext(nc) as tc, ExitStack() as ctx:
        const = ctx.enter_context(tc.tile_pool(name="const", bufs=1))
        fpool = ctx.enter_context(tc.tile_pool(name="fts", bufs=4))
        rawpool = ctx.enter_context(tc.tile_pool(name="raws", bufs=4))
        gpool = ctx.enter_context(tc.tile_pool(name="grp", bufs=2))
        spool = ctx.enter_context(tc.tile_pool(name="small", bufs=2))
        raw_ps = ctx.enter_context(tc.tile_pool(name="rawps", bufs=6, space="PSUM"))

        # Constants (off the sync queue so the first feature block leads it)
        rt_sb = const.tile([P, KC, 2 * DM], F16)
        nc.scalar.dma_start(rt_sb[:], rt2[:, :].rearrange("(k p) j -> p k j", p=P))
        r_all = const.tile([P, NT], F32)
        nc.scalar.dma_start(r_all[:], rin[:, :])
        bias_shift = const.tile([P, 1], F32)
        nc.gpsimd.memset(bias_shift[:], -SHIFT)
        out_sb = const.tile([P, NT, D], F32)

        fhi_v = fhi[:, :].rearrange("(k p) b -> p k b", p=P)
        flo_v = flo[:, :].rearrange("(k p) b -> p k b", p=P)

        for g in range(NG * repeat):
            g = g % NG
            ex_g = gpool.tile([P, G, D * M], F32, tag="exg")
            t_g = gpool.tile([P, G, D * M], F32, tag="tg")

            # Loads: 4-tile blocks, alternating DMA issuers. The first group
            # uses single-tile blocks so the first matmul starts ~4x sooner.
            lb = P if g == 0 else LB
            hi_blocks, lo_blocks = [], []
            for h in range(G * P // lb):
                t0 = g * G * P + h * lb
                hi_sb = fpool.tile([P, KC, lb], F16, tag=f"fhi{min(g,1)}")
                lo_sb = fpool.tile([P, KC, lb], F16, tag=f"flo{min(g,1)}")
                eng_a = nc.sync if h % 2 == 0 else nc.gpsimd
                eng_b = nc.gpsimd if h % 2 == 0 else nc.sync
                eng_a.dma_start(hi_sb[:], fhi_v[:, :, t0 : t0 + lb])
                eng_b.dma_start(lo_sb[:], flo_v[:, :, t0 : t0 + lb])
                hi_blocks.append(hi_sb)
                lo_blocks.append(lo_sb)

            for s in range(G):
                t = g * G + s
                blk = s * P // lb
                sl = slice((s % (lb // P)) * P, (s % (lb // P) + 1) * P)
                hi_sb, lo_sb = hi_blocks[blk], lo_blocks[blk]
                raw = raw_ps.tile([P, DM], F32, tag="raw")
                for k in range(KC):
                    # raw += fhi@Rhi + fhi@Rlo + flo@Rhi  (all into one bank)
                    nc.tensor.matmul(
                        raw[:], hi_sb[:, k, sl], rt_sb[:, k, 0:DM],
                        start=(k == 0), stop=False,
                    )
                    nc.tensor.matmul(
                        raw[:], hi_sb[:, k, sl], rt_sb[:, k, DM : 2 * DM],
                        start=False, stop=False,
                    )
                    nc.tensor.matmul(
                        raw[:], lo_sb[:, k, sl], rt_sb[:, k, 0:DM],
                        start=False, stop=(k == KC - 1),
                    )
                nc.scalar.activation(
                    ex_g[:, s, :],
                    raw[:, 0 : D * M],
                    AF.Exp,
                    bias=bias_shift[:],
                    scale=r_all[:, t : t + 1],
                )
                nc.scalar.copy(t_g[:, s, :], raw[:, D * M : DM])

            # Grouped softmax tail
            sums = spool.tile([P, G, D], F32, tag="sums")
            nc.vector.reduce_sum(
                sums[:],
                ex_g[:].rearrange("p s (d m) -> p s d m", d=D, m=M),
                axis=mybir.AxisListType.X,
            )
            prod = spool.tile([P, G, D * M], F32, tag="prod")
            nc.vector.tensor_mul(prod[:], ex_g[:], t_g[:])
            wsum = spool.tile([P, G, D], F32, tag="wsum")
            nc.vector.reduce_sum(
                wsum[:],
                prod[:].rearrange("p s (d m) -> p s d m", d=D, m=M),
                axis=mybir.AxisListType.X,
            )
            rsums = spool.tile([P, G, D], F32, tag="rsums")
            nc.vector.reciprocal(rsums[:], sums[:])
            datt0 = spool.tile([P, G, D], F32, tag="datt0")
            nc.vector.tensor_mul(datt0[:], wsum[:], rsums[:])
            datt = spool.tile([P, G, D], F32, tag="datt")
            rg = r_all[:, g * G : (g + 1) * G]
            nc.vector.tensor_mul(
                datt[:], datt0[:], rg[:, :, None].broadcast_to([P, G, D])
            )
            ex2 = spool.tile([P, G, D], F32, tag="ex2")
            nc.scalar.activation(ex2[:], datt[:], AF.Exp, bias=bias_shift[:])
            sumd = spool.tile([P, G], F32, tag="sumd")
            nc.vector.reduce_sum(sumd[:], ex2[:], axis=mybir.AxisListType.X)
            rd = spool.tile([P, G], F32, tag="rd")
            nc.vector.reciprocal(rd[:], sumd[:])
            nc.vector.tensor_mul(
                out_sb[:, g * G : (g + 1) * G, :],
                ex2[:],
                rd[:, :, None].broadcast_to([P, G, D]),
            )

            out_v = out[:, :].rearrange("(t p) d -> p t d", p=P)
            nc.sync.dma_start(
                out_v[:, g * G : (g + 1) * G, :], out_sb[:, g * G : (g + 1) * G, :]
            )

    # All ACT functions used (Exp, Copy/Identity) live in one table set; steer
    # the table-load placement pass to a single covering set to avoid
    # alternating ~2.7us table loads.
    mine = {AF.Exp, AF.Ln, AF.Square, AF.Copy, AF.Identity}
    orig_tables = bacc.get_activation_tables

    def _patched(arch):
        return {
            name: (fns if name == "natural_log_exp_and_others" else fns - mine)
            for name, fns in orig_tables(arch).items()
        }

    bacc.get_activation_tables = _patched
    try:
        nc.finalize()
    finally:
        bacc.get_activation_tables = orig_tables
    return nc


def _get_nc():
    if "nc" not in _CACHE:
        _CACHE["nc"] = _build_nc()
    return _CACHE["nc"]


def _host_prep(feature, W_topic, W_domain, memory):
    """R matrix, bf16 splits and per-row scale factors, per core."""
    BF = np.float16
    mem_flat = memory.reshape(D * M, E).astype(np.float64)
    Pm = mem_flat @ W_topic.astype(np.float64)
    Qm = mem_flat @ W_domain.astype(np.float64)
    R = np.concatenate([Pm, Qm], axis=0).astype(np.float32)  # [180, 768]
    Rhi = R.astype(BF)
    Rlo = (R - Rhi.astype(np.float32)).astype(BF)
    rt2 = np.concatenate([Rhi.T, Rlo.T], axis=1)  # [768, 360] bf16
    rt2 = np.ascontiguousarray(rt2)

    f = np.asarray(feature, dtype=np.float32)
    norm2 = (f.astype(np.float64) ** 2).sum(axis=1)
    r_rows = (TAU / np.sqrt(norm2)).astype(np.float32)  # [B]

    per_core = []
    for c in range(NCORES):
        fc = f[c * BC : (c + 1) * BC]
        ft = np.ascontiguousarray(fc.T)  # [768, BC] f32
        fhi = ft.astype(BF)
        flo = (ft - fhi.astype(np.float32)).astype(BF)
        rin = np.ascontiguousarray(
            r_rows[c * BC : (c + 1) * BC].reshape(NT, P).T
        )  # [P, NT]
        per_core.append(
            {"fhi": fhi, "flo": flo, "rt2": rt2, "rin": rin}
        )
    return per_core


def kernel(feature, category, W_topic, W_domain, memory):
    from concourse.bass_utils import run_bass_kernel_spmd

    in_maps = _host_prep(
        feature, np.asarray(W_topic), np.asarray(W_domain), np.asarray(memory)
    )
    nc = _get_nc()
    res = run_bass_kernel_spmd(nc, in_maps, core_ids=list(range(NCORES)))
    outs = [res.results[c]["out"] for c in range(NCORES)]
    full = np.concatenate(outs, axis=0)  # [B, 9]
    return full[:, None, :].astype(np.float32)



# revision 7
# speedup vs baseline: 1.2493x; 1.2493x over previous
"""Trainium2 Bass kernel for nn_MemoryNetwork (scatter_memory).

Computation (reference, per batch row b):
    f = feature / ||feature||                       [B, 768]
    topic = f @ W_topic.T ; dom = f @ W_domain.T    [B, 256]
    att   = softmax_m(TAU * topic . memory[d,m])    [B, 9, 10]
    sep   = sum_m att * memory[d,m]                 [B, 9, 256]
    out   = softmax_d(TAU * sep . dom)              [B, 1, 9]

Reformulation: fold the tiny memory banks into the projections on host:
    R = [mem_flat @ W_topic; mem_flat @ W_domain]   [180, 768]
and pre-scale rows by r = TAU/||f|| (host): fs = r*f. Then per row
    raw  = fs @ R.T          (rawS = raw[:90], rawT = raw[90:])
    ex   = exp(rawS - SHIFT) (const shift; logits in [-130, 110])
    datt = (sum_m ex*rawT) / (sum_m ex)   per domain
    out  = softmax_d(datt)   (const shift again)

Precision: fs and R are split fp16 + residual; residual products run as
fp8 e5m2 in DoubleRow mode (2 k-tiles per instruction, 2x rate):
    raw = fhi16@Rhi16 (6 fp16 matmuls, k-chunks)
        + fhi8@Rlo8 + flo8@Rhi8 (6 DoubleRow fp8 matmuls)
fhi8 is produced on-device by a casting SBUF->SBUF DMA (gpsimd queue,
round-to-nearest, bit-exact vs ml_dtypes). Measured end-to-end error
~4e-3 vs the fp32 reference (gate 2e-2).

Sharding: data-parallel over B across 8 cores (4096 rows each). Features
are shipped pre-tiled [128, NT, KC, 128] (hi fp16 + lo fp8e5m2, 3 B/elem
= 9.4 MB/core), so each DMA block is contiguous per partition.

Schedule: half-groups of 4 batch tiles accumulate into one 4-bank PSUM
tile (ping-pong, 8 banks total); the exp/copy epilogue runs as batched
ACT ops across the 4 banks; the softmax tail runs as grouped DVE ops over
8 tiles (final two half-groups run solo to shorten the post-matmul tail).
"""

import sys

sys.path.insert(0, "/opt/trn_rl_repo")

import numpy as np
import ml_dtypes

B, IN, E, D, M = 32768, 768, 256, 9, 10
NCORES = 8
BC = B // NCORES   # rows per core
P = 128            # partition tile
NT = BC // P       # batch tiles per core (32)
KC = IN // P       # contraction chunks (6)
DM = 2 * D * M     # 180
TAU = 32.0
SHIFT = 50.0
H = NT // 4        # half-groups of 4 tiles (8)
# epilogue groups as lists of half-group indices
EPI_GROUPS = [[0, 1], [2, 3], [4, 5], [6], [7]]

_CACHE: dict = {}


def _build_nc(repeat=1):
    from contextlib import ExitStack

    import concourse.bacc as bacc
    import concourse.tile as tile
    from concourse import mybir

    F32 = mybir.dt.float32
    F16 = mybir.dt.float16
    E5 = mybir.dt.float8e5
    AF = mybir.ActivationFunctionType
    DR = mybir.MatmulPerfMode.DoubleRow
    AX = mybir.AxisListType.X

    nc = bacc.Bacc(trn_type="TRN2")
    fhi = nc.dram_tensor("fhi", [P, NT, KC, P], F16, kind="ExternalInput")
    flo = nc.dram_tensor("flo", [P, NT, KC, P], E5, kind="ExternalInput")
    rt16 = nc.dram_tensor("rt16", [P, KC, 2, DM], F16, kind="ExternalInput")
    rt8 = nc.dram_tensor("rt8", [P, KC, 2, DM], E5, kind="ExternalInput")
    out = nc.dram_tensor("out", [BC, D], F32, kind="ExternalOutput")

    with tile.TileContext(nc) as tc, ExitStack() as ctx:
        const = ctx.enter_context(tc.tile_pool(name="const", bufs=1))
        fpool = ctx.enter_context(tc.tile_pool(name="fts", bufs=7))
        gpool = ctx.enter_context(tc.tile_pool(name="grp", bufs=2))
        spool = ctx.enter_context(tc.tile_pool(name="small", bufs=2))
        raw_ps = ctx.enter_context(tc.tile_pool(name="rawps", bufs=2, space="PSUM"))

        # Constants first on the scalar queue so R lands before matmul 0.
        rt16_sb = const.tile([P, KC, 2, DM], F16)
        nc.scalar.dma_start(rt16_sb[:], rt16[:, :, :, :])
        rt8_sb = const.tile([P, KC, 2, DM], E5)
        nc.scalar.dma_start(rt8_sb[:], rt8[:, :, :, :])
        bias_shift = const.tile([P, 1], F32)
        nc.gpsimd.memset(bias_shift[:], -SHIFT)

        out_v = out[:, :].rearrange("(t p) d -> p t d", p=P)

        for it in range(repeat):
            # Feature DMA blocks: 4 single-tile blocks (fast start), then
            # 7 quad blocks. hi on sync, lo on scalar, fp8 cast on gpsimd.
            hi_blocks, lo_blocks, hi8_blocks = [], [], []
            for bi in range(11):
                t0, bn = (bi, 1) if bi < 4 else (4 + (bi - 4) * 4, 4)
                hi_sb = fpool.tile([P, bn, KC, P], F16, tag=f"fhi{bn}")
                lo_sb = fpool.tile([P, bn, KC, P], E5, tag=f"flo{bn}")
                hi8_sb = fpool.tile([P, bn, KC, P], E5, tag=f"fhi8{bn}")
                nc.sync.dma_start(hi_sb[:], fhi[:, t0 : t0 + bn, :, :])
                nc.scalar.dma_start(lo_sb[:], flo[:, t0 : t0 + bn, :, :])
                nc.gpsimd.dma_start(hi8_sb[:], hi_sb[:, :, :, :])
                hi_blocks.append((t0, bn, hi_sb))
                lo_blocks.append(lo_sb)
                hi8_blocks.append(hi8_sb)

            def tile_view(blocks, s):
                for i, (t0, bn, _) in enumerate(hi_blocks):
                    if t0 <= s < t0 + bn:
                        return blocks[i][:, s - t0]
                raise AssertionError

            raw_halves = []
            for h in range(H):
                raw4 = raw_ps.tile([P, 4, 512], F32, tag="raw4")
                raw_halves.append(raw4)
                for j in range(4):
                    s = 4 * h + j
                    hi_t = tile_view([b for _, _, b in hi_blocks], s)
                    lo_t = tile_view(lo_blocks, s)
                    hi8_t = tile_view(hi8_blocks, s)
                    acc = raw4[:, j, 0:DM]
                    for k in range(KC):
                        nc.tensor.matmul(
                            acc, hi_t[:, k, :], rt16_sb[:, k, 0, :],
                            start=(k == 0), stop=False,
                        )
                    for j2 in range(KC // 2):
                        kk = slice(2 * j2, 2 * j2 + 2)
                        nc.tensor.matmul(
                            acc, lo_t[:, kk, :], rt8_sb[:, kk, 0, :],
                            start=False, stop=False, perf_mode=DR,
                        )
                    for j2 in range(KC // 2):
                        kk = slice(2 * j2, 2 * j2 + 2)
                        nc.tensor.matmul(
                            acc, hi8_t[:, kk, :], rt8_sb[:, kk, 1, :],
                            start=False, stop=(j2 == KC // 2 - 1), perf_mode=DR,
                        )

            for gi, halves in enumerate(EPI_GROUPS):
                G = 4 * len(halves)
                t0 = 4 * halves[0]
                ex_g = gpool.tile([P, G, D * M], F32, tag=f"exg{len(halves)}")
                t_g = gpool.tile([P, G, D * M], F32, tag=f"tg{len(halves)}")
                for hj, h in enumerate(halves):
                    raw4 = raw_halves[h]
                    sl = slice(4 * hj, 4 * hj + 4)
                    nc.scalar.activation(
                        ex_g[:, sl, :], raw4[:, :, 0 : D * M], AF.Exp,
                        bias=bias_shift[:],
                    )
                    nc.scalar.copy(t_g[:, sl, :], raw4[:, :, D * M : DM])

                sums = spool.tile([P, G, D], F32, tag=f"sums{len(halves)}")
                nc.vector.reduce_sum(
                    sums[:],
                    ex_g[:].rearrange("p s (d m) -> p s d m", d=D, m=M),
                    axis=AX,
                )
                prod = spool.tile([P, G, D * M], F32, tag=f"prod{len(halves)}")
                nc.vector.tensor_mul(prod[:], ex_g[:], t_g[:])
                wsum = spool.tile([P, G, D], F32, tag=f"wsum{len(halves)}")
                nc.vector.reduce_sum(
                    wsum[:],
                    prod[:].rearrange("p s (d m) -> p s d m", d=D, m=M),
                    axis=AX,
                )
                rsums = spool.tile([P, G, D], F32, tag=f"rsums{len(halves)}")
                nc.vector.reciprocal(rsums[:], sums[:])
                datt = spool.tile([P, G, D], F32, tag=f"datt{len(halves)}")
                nc.vector.tensor_mul(datt[:], wsum[:], rsums[:])
                ex2 = spool.tile([P, G, D], F32, tag=f"ex2{len(halves)}")
                nc.scalar.activation(ex2[:], datt[:], AF.Exp, bias=bias_shift[:])
                sumd = spool.tile([P, G], F32, tag=f"sumd{len(halves)}")
                nc.vector.reduce_sum(sumd[:], ex2[:], axis=AX)
                rd = spool.tile([P, G], F32, tag=f"rd{len(halves)}")
                nc.vector.reciprocal(rd[:], sumd[:])
                out_t = spool.tile([P, G, D], F32, tag=f"outt{len(halves)}")
                nc.vector.tensor_mul(
                    out_t[:], ex2[:], rd[:, :, None].broadcast_to([P, G, D])
                )
                nc.sync.dma_start(out_v[:, t0 : t0 + G, :], out_t[:])

    # All ACT functions used (Exp, Copy/Identity) live in one table set; steer
    # the table-load placement pass to a single covering set to avoid
    # alternating ~2.7us table loads.
    mine = {AF.Exp, AF.Ln, AF.Square, AF.Copy, AF.Identity}
    orig_tables = bacc.get_activation_tables

    def _patched(arch):
        return {
            name: (fns if name == "natural_log_exp_and_others" else fns - mine)
            for name, fns in orig_tables(arch).items()
        }

    bacc.get_activation_tables = _patched
    try:
        nc.finalize()
    finally:
        bacc.get_activation_tables = orig_tables
    return nc


def _get_nc():
    if "nc" not in _CACHE:
        _CACHE["nc"] = _build_nc()
    return _CACHE["nc"]


def _host_prep(feature, W_topic, W_domain, memory):
    """R splits and per-core pre-scaled, pre-tiled feature splits."""
    E5np = ml_dtypes.float8_e5m2
    mem_flat = memory.reshape(D * M, E).astype(np.float64)
    Pm = mem_flat @ W_topic.astype(np.float64)
    Qm = mem_flat @ W_domain.astype(np.float64)
    RT = np.concatenate([Pm, Qm], axis=0).T.astype(np.float32)  # [768, 180]
    RhiT = RT.astype(np.float16)
    RloT = RT - RhiT.astype(np.float32)
    # [P, KC, 2, DM] with [:, k, 0] = Rhi chunk k, [:, k, 1] = Rlo chunk k
    rt16 = np.ascontiguousarray(
        np.stack([RhiT.astype(np.float16).reshape(KC, P, DM),
                  RloT.astype(np.float16).reshape(KC, P, DM)],
                 axis=2).transpose(1, 0, 2, 3))
    rt8 = rt16.astype(E5np)

    f = np.asarray(feature, dtype=np.float64)
    r = TAU / np.sqrt((f ** 2).sum(axis=1))
    fs = (f * r[:, None]).astype(np.float32)

    per_core = []
    for c in range(NCORES):
        ft = fs[c * BC : (c + 1) * BC].T  # [768, BC]
        # [P, NT, KC, P]: tile t, chunk k, col c -> ft[k*128+p, t*128+c]
        tiled = ft.reshape(KC, P, NT, P).transpose(1, 2, 0, 3)
        fhi = np.ascontiguousarray(tiled.astype(np.float16))
        flo = np.ascontiguousarray(
            (tiled - fhi.astype(np.float32)).astype(E5np))
        per_core.append({"fhi": fhi, "flo": flo, "rt16": rt16, "rt8": rt8})
    return per_core


def kernel(feature, category, W_topic, W_domain, memory):
    from concourse.bass_utils import run_bass_kernel_spmd

    in_maps = _host_prep(
        feature, np.asarray(W_topic), np.asarray(W_domain), np.asarray(memory)
    )
    nc = _get_nc()
    res = run_bass_kernel_spmd(nc, in_maps, core_ids=list(range(NCORES)))
    outs = [res.results[c]["out"] for c in range(NCORES)]
    full = np.concatenate(outs, axis=0)  # [B, 9]
    return full[:, None, :].astype(np.float32)
